# revision 24
# baseline (speedup 1.0000x reference)
"""Trainium2 Bass kernel: batched Butterworth bandpass filtfilt (order 4).

Implements y = filtfilt(b, a, x) per batch row (scipy default odd padding),
exactly mirroring the reference: odd-extend by 27, forward IIR pass with
zi*ext[0] initial state, backward (anti-causal) pass with zi*y1[-1], slice.

Algorithm (per core, 128 batch rows in the partition dim):
  The DF2T recurrence z_{t+1} = A z_t + g x_t, y_t = b0 x_t + z_t[0] is
  block-parallelized: time is split into 129 tiles of L=128 samples on a grid
  left-padded by 74 zeros (so both passes share one block-aligned grid).
  Per tile: y_tile = Htri @ x_tile + P @ z_tile (Toeplitz matmul + state
  injection). States at all 16 tile starts of a chunk are produced stacked in
  one PSUM tile by 17 accumulating matmuls whose lhsT are column slices of one
  wide constant [zeros | Q0^T]. Pass 1 repartitions the stack to [8, 16*128]
  (via a DRAM bounce) so four tiles' states form one N=512 moving operand;
  pass 2 consumes 32-row slices of the stack directly as stationary operands
  against a block-diagonal constant. The backward pass is the same machinery
  with time-reversed constants, consuming the forward pass's time-major
  output tiles directly (no data flips anywhere).

All constant matrices are built host-side in float64 from the b/a/zi inputs
and stored fp32. Data-parallel across 8 NeuronCores (batch sharding).
"""
import ml_dtypes
import numpy as np
from contextlib import ExitStack

import concourse.bass as bass  # noqa: F401
import concourse.tile as tile
from concourse import bacc, mybir
from concourse.bass_utils import run_bass_kernel_spmd

BATCH, T = 1024, 16384
NCORES = 8
BSH = BATCH // NCORES
L = 128
S = 8
CH = 16
PADLEN = 27
T0 = T + 2 * PADLEN          # 16438
PADL = (-T0) % L             # 74
TP = T0 + PADL               # 16512
NB = TP // L                 # 129
NG = (NB + 3) // 4           # 33 groups of <=4 tiles (last has 1)
NCHUNK = 8                   # full 16-tile chunks; tile 128 rides on chunk 7's tail
F32 = mybir.dt.float32
BF16 = mybir.dt.bfloat16

CONST_SHAPES = {
    "HT1": (L, L), "PT1": (S, L), "QT1w": (L, 2 * L), "SST": (S, L),
    "HT2": (L, L), "QT2w": (L, 2 * L),
    "PG2_0": (L, 512), "PG2_1": (L, 512), "PG2_2": (L, 512), "PG2_3": (L, 512),
    "injZ": (1, L), "injy0": (1, L), "s2z": (1, L), "p2zi": (1, L),
}


def _build_const_arrays(b, a, zi):
    """Ascending stacking: stacked row-block rho holds the state entering
    chunk block rho+1 (rho=15 is the chunk-chain output)."""
    b = np.asarray(b, np.float64)
    a = np.asarray(a, np.float64)
    zi = np.asarray(zi, np.float64)
    A = np.zeros((S, S))
    A[:, 0] = -a[1:]
    A[np.arange(S - 1), np.arange(1, S)] = 1.0
    g = b[1:] - a[1:] * b[0]
    Apow = [np.eye(S)]
    for _ in range(L):
        Apow.append(A @ Apow[-1])
    A_L = Apow[L]
    ALp = [np.eye(S)]
    for _ in range(CH + 1):
        ALp.append(A_L @ ALp[-1])
    h = np.zeros(L)
    h[0] = b[0]
    for m in range(1, L):
        h[m] = (Apow[m - 1] @ g)[0]
    W = np.stack([Apow[L - 1 - j] @ g for j in range(L)], axis=1)   # [S, L]
    Htri = np.zeros((L, L))
    for j in range(L):
        Htri[j, :j + 1] = h[j::-1]
    P = np.stack([Apow[j][0, :] for j in range(L)], axis=0)         # [L, S]
    Jf = np.eye(L)[::-1]
    Htri2 = Jf @ Htri @ Jf
    P2 = Jf @ P
    W2 = W @ Jf

    Q0 = np.zeros((L, L))       # row 8*rho+s = (A_L^rho W)[s]
    Q02 = np.zeros((L, L))
    Sst = np.zeros((L, S))      # row 8*rho+s = (A_L^{rho+1})[s]
    injZ = np.zeros(L)          # pass-1 chunk-0 seed: (A_L^rho @ zint)[s]
    s2z = np.zeros(L)           # pass-2 chunk-0 seed: (A_L^{rho+1} zi)[s]
    zint = Apow[L - PADL] @ zi
    for rho in range(CH):
        r0 = rho * S
        Q0[r0:r0 + S, :] = ALp[rho] @ W
        Q02[r0:r0 + S, :] = ALp[rho] @ W2
        Sst[r0:r0 + S, :] = ALp[rho + 1]
        injZ[r0:r0 + S] = ALp[rho] @ zint
        s2z[r0:r0 + S] = ALp[rho + 1] @ zi
    QT1w = np.concatenate([np.zeros((L, L)), Q0.T], axis=1)          # [L, 2L]
    QT2w = np.concatenate([np.zeros((L, L)), Q02.T], axis=1)
    injy0 = np.zeros(L)
    for j in range(PADL, L):
        injy0[j] = (Apow[j - PADL] @ zi)[0]
    p2zi = P2 @ zi
    # PG2_k [128, 512]: contracting the full stacked-state tile against a
    # block-diagonal selector: row 8*rho+sigma feeds out col-slice i iff
    # rho == 4k + 3 - i (ascending rho = descending tile within a group).
    PG2 = [np.zeros((L, 512)) for _ in range(4)]
    for k in range(4):
        for i_out in range(4):
            rho = 4 * k + 3 - i_out
            PG2[k][S * rho:S * (rho + 1), 128 * i_out:128 * (i_out + 1)] = P2.T
    f = ml_dtypes.bfloat16
    return {
        "HT1": np.ascontiguousarray(Htri.T, f), "PT1": np.ascontiguousarray(P.T, f),
        "QT1w": np.ascontiguousarray(QT1w, f), "SST": np.ascontiguousarray(Sst.T, f),
        "HT2": np.ascontiguousarray(Htri2.T, f), "QT2w": np.ascontiguousarray(QT2w, f),
        "PG2_0": PG2[0].astype(f), "PG2_1": PG2[1].astype(f),
        "PG2_2": PG2[2].astype(f), "PG2_3": PG2[3].astype(f),
        "injZ": injZ.reshape(1, L).astype(f), "injy0": injy0.reshape(1, L).astype(f),
        "s2z": s2z.reshape(1, L).astype(f), "p2zi": p2zi.reshape(1, L).astype(f),
    }


_NC_CACHE = None


def _build_nc():
    nc = bacc.Bacc("TRN2", target_bir_lowering=False, debug=False,
                   num_devices=NCORES)
    xe = nc.dram_tensor("xe", [BSH, TP], BF16, kind="ExternalInput").ap()
    yo = nc.dram_tensor("y", [BSH, T], F32, kind="ExternalOutput").ap()
    cdr = {n: nc.dram_tensor(n, list(s), BF16, kind="ExternalInput").ap()
           for n, s in CONST_SHAPES.items()}

    with tile.TileContext(nc) as tc, ExitStack() as ctx:
        consts = ctx.enter_context(tc.tile_pool(name="consts", bufs=1))
        xtpool = ctx.enter_context(tc.tile_pool(name="xtp", bufs=1))
        y1pool = ctx.enter_context(tc.tile_pool(name="y1p", bufs=1))
        zcpool = ctx.enter_context(tc.tile_pool(name="zcp", bufs=1))
        ystage = ctx.enter_context(tc.tile_pool(name="ystage", bufs=3))
        dstage = ctx.enter_context(tc.tile_pool(name="dstage", bufs=2, space="DRAM"))
        ps_zc = ctx.enter_context(tc.tile_pool(name="ps_zc", bufs=3, space="PSUM"))
        ps_y = ctx.enter_context(tc.tile_pool(name="ps_y", bufs=4, space="PSUM"))

        C = {}
        for n, s in CONST_SHAPES.items():
            t = consts.tile(list(s), BF16, tag=f"c_{n}", name=f"sb_{n}")
            nc.scalar.dma_start(out=t, in_=cdr[n])
            C[n] = t
        # ext[:,0] (= xe col PADL) as a [1, BSH] partition-0 row
        ext0_t = consts.tile([1, BSH], BF16, tag="ext0")
        nc.gpsimd.dma_start(out=ext0_t, in_=xe[:, PADL:PADL + 1].rearrange("b one -> one b"))

        # ---------------- PASS 1 ----------------
        # X^T tiles loaded directly from DRAM via xbar DMA-transpose (bf16)
        xt_tiles = []
        for gi in range(NG):
            nt = 4 if gi < NG - 1 else 1
            xt = xtpool.tile([L, 512], BF16, tag=f"xt{gi}", name=f"xt{gi}")
            for i in range(nt):
                q = 4 * gi + i
                nc.sync.dma_start(out=xt[:, i * L:(i + 1) * L],
                                  in_=xe[:, q * L:(q + 1) * L], transpose=True)
            xt_tiles.append(xt)

        def XT(q):
            return xt_tiles[q // 4][:, (q % 4) * L:(q % 4 + 1) * L]

        def chunk_stack(tag, qtw, rhs_tile_fn, seed_lhsT, seed_rhs, zchain):
            """17 accumulating matmuls -> stacked states [128, BSH] in PSUM;
            copies to SBUF and returns (stack, chain) where chain is the
            rows [120:128] repositioned to partition 0 via a small DMA."""
            zp = ps_zc.tile([L, BSH], F32, tag="zp", name=f"zp_{tag}")
            for m in range(CH):
                nc.tensor.matmul(zp, qtw[:, L - S * m:2 * L - S * m],
                                 rhs_tile_fn(m), start=(m == 0), stop=False)
            if zchain is None:
                nc.tensor.matmul(zp, seed_lhsT, seed_rhs, start=False, stop=True)
            else:
                nc.tensor.matmul(zp, C["SST"], zchain, start=False, stop=True)
            zs = zcpool.tile([L, BSH], BF16, tag=f"zs{tag[0]}", bufs=2, name=f"zs_{tag}")
            nc.scalar.copy(out=zs, in_=zp)
            zch = zcpool.tile([S, BSH], BF16, tag=f"zch{tag[0]}", bufs=2, name=f"zch_{tag}")
            nc.gpsimd.dma_start(out=zch, in_=zs[L - S:L, :])
            return zs, zch

        zc1 = []   # pass-1 repartitioned states [8, CH*BSH]: col-block u = state of tile 16c+u+1
        zch1 = []
        for c in range(NCHUNK):
            zchain = None if c == 0 else zch1[c - 1]
            zs, zch = chunk_stack(f"1_{c}", C["QT1w"], lambda m, c=c: XT(16 * c + m),
                                  C["injZ"], ext0_t, zchain)
            zch1.append(zch)
            dz = dstage.tile([CH, S, BSH], BF16, tag="dz", name=f"dz{c}")
            nc.scalar.dma_start(out=dz, in_=zs)
            zr = zcpool.tile([S, CH * BSH], BF16, tag="zr1", bufs=3, name=f"zr1_{c}")
            nc.scalar.dma_start(out=zr.rearrange("s (r b) -> s r b", r=CH),
                                in_=dz.rearrange("r s b -> s r b"))
            zc1.append(zr)

        y1_tiles = [None] * NG

        def emit_y1_group(gi):
            nt = 4 if gi < NG - 1 else 1
            ncol = nt * L
            yp = ps_y.tile([L, 512], F32, tag="yp", name=f"yp1_{gi}")
            nc.tensor.matmul(yp[:, :ncol], C["HT1"], xt_tiles[gi][:, :ncol],
                             start=True, stop=False)
            # states for tiles 4gi..4gi+3 live at u = (q-1) mod 16 in chunk (q-1)//16
            q0 = 4 * gi
            if gi == 0:
                nc.tensor.matmul(yp[:, 0:L], C["injy0"], ext0_t,
                                 start=False, stop=False)
                nc.tensor.matmul(yp[:, L:4 * L], C["PT1"], zc1[0][:, 0:3 * BSH],
                                 start=False, stop=True)
            elif gi == NG - 1:
                nc.tensor.matmul(yp[:, 0:L], C["PT1"],
                                 zc1[7][:, (CH - 1) * BSH:CH * BSH],
                                 start=False, stop=True)
            elif gi % 4 == 0:
                c = gi // 4
                nc.tensor.matmul(yp[:, 0:L], C["PT1"],
                                 zc1[c - 1][:, (CH - 1) * BSH:CH * BSH],
                                 start=False, stop=False)
                nc.tensor.matmul(yp[:, L:4 * L], C["PT1"], zc1[c][:, 0:3 * BSH],
                                 start=False, stop=True)
            else:
                c, u0 = divmod(q0 - 1, CH)
                nc.tensor.matmul(yp[:, :4 * L], C["PT1"],
                                 zc1[c][:, u0 * BSH:(u0 + 4) * BSH],
                                 start=False, stop=True)
            y1 = y1pool.tile([L, 512], BF16, tag=f"y1{gi}", name=f"y1_{gi}")
            nc.vector.tensor_copy(out=y1[:, :ncol], in_=yp[:, :ncol])
            y1_tiles[gi] = y1

        for c in range(NCHUNK):
            for gi in range(4 * c, 4 * c + 4):
                emit_y1_group(gi)
        emit_y1_group(NG - 1)

        def Y1T(q):
            return y1_tiles[q // 4][:, (q % 4) * L:(q % 4 + 1) * L]

        # y1[:, t=16437] as a [1, BSH] partition-0 row (via SBUF->SBUF DMA)
        w0_t = consts.tile([1, BSH], BF16, tag="w0")
        nc.gpsimd.dma_start(out=w0_t, in_=Y1T(NB - 1)[L - 1:L, :])

        # ---------------- PASS 2 (right-to-left) ----------------
        zc2 = []     # stacked SBUF states per chunk [128, BSH]
        zch2 = []    # chain rows [120:128] repositioned to partition 0

        def emit_y2_group(gi):
            # tiles 4gi..4gi+3; their states: chunk c rows [8*rho0, +32),
            # rho0 = 124 - 16c - 4gi, in reversed within-group order (PG2).
            nt = 4 if gi < NG - 1 else 1
            yp = ps_y.tile([BSH, 512], F32, tag="yp", name=f"yp2_{gi}")
            if gi == NG - 1:
                nc.tensor.matmul(yp[:, 0:L], w0_t, C["p2zi"], start=True, stop=False)
                nc.tensor.matmul(yp[:, 0:L], Y1T(NB - 1), C["HT2"],
                                 start=False, stop=True)
            else:
                c = (127 - 4 * gi - 3) // 16
                k = (124 - 16 * c - 4 * gi) // 4
                nc.tensor.matmul(yp, zc2[c], C[f"PG2_{k}"], start=True, stop=False)
                for i in range(nt):
                    q = 4 * gi + i
                    nc.tensor.matmul(yp[:, i * L:(i + 1) * L], Y1T(q), C["HT2"],
                                     start=False, stop=(i == nt - 1))
            ys = ystage.tile([BSH, 512], F32, tag="ys", name=f"ys{gi}")
            nc.vector.tensor_copy(out=ys[:, :nt * L], in_=yp[:, :nt * L])
            # tile q covers y-out index range [128q - 101, 128q + 27)
            if gi == 0:
                nc.scalar.dma_start(out=yo[:, 0:411], in_=ys[:, 101:512])
            elif gi == NG - 1:
                nc.scalar.dma_start(out=yo[:, 512 * gi - 101:T], in_=ys[:, 0:101])
            else:
                nc.scalar.dma_start(out=yo[:, 512 * gi - 101:512 * gi + 411],
                                    in_=ys[:, 0:512])

        emit_y2_group(NG - 1)
        for c in range(NCHUNK):
            zchain = None if c == 0 else zch2[c - 1]
            zs, zch = chunk_stack(f"2_{c}", C["QT2w"],
                                  lambda m, c=c: Y1T(NB - 1 - 16 * c - m),
                                  C["s2z"], w0_t, zchain)
            zc2.append(zs)
            zch2.append(zch)
            for gi in range(31 - 4 * c, 27 - 4 * c, -1):
                emit_y2_group(gi)
    nc.compile()
    return nc


def _host_ext(x):
    x = np.asarray(x, np.float32)
    left = 2.0 * x[:, :1] - x[:, 1:PADLEN + 1][:, ::-1]
    right = 2.0 * x[:, -1:] - x[:, -PADLEN - 1:-1][:, ::-1]
    ext = np.concatenate(
        [np.zeros((x.shape[0], PADL), np.float32), left, x, right], axis=1)
    return ext.astype(ml_dtypes.bfloat16)


def _run(x, b, a, zi, **spmd_kwargs):
    global _NC_CACHE
    if _NC_CACHE is None:
        _NC_CACHE = _build_nc()
    nc = _NC_CACHE
    consts = _build_const_arrays(b, a, zi)
    ext = _host_ext(x)
    in_maps = []
    for i in range(NCORES):
        m = {"xe": np.ascontiguousarray(ext[i * BSH:(i + 1) * BSH])}
        m.update(consts)
        in_maps.append(m)
    res = run_bass_kernel_spmd(nc, in_maps, core_ids=list(range(NCORES)),
                               **spmd_kwargs)
    out = np.concatenate([res.results[i]["y"] for i in range(NCORES)], axis=0)
    return out, res


def kernel(x, b, a, zi):
    return _run(x, b, a, zi)[0]


# revision 25
# speedup vs baseline: 2.0601x; 2.0601x over previous
"""Trainium2 Bass kernel: batched Butterworth bandpass filtfilt (order 4).

Implements y = filtfilt(b, a, x) per batch row (scipy default odd padding),
exactly mirroring the reference: odd-extend by 27, forward IIR pass with
zi*ext[0] initial state, backward (anti-causal) pass with zi*y1[-1], slice.

Algorithm (per core, 128 batch rows in the partition dim):
  The DF2T recurrence z_{t+1} = A z_t + g x_t, y_t = b0 x_t + z_t[0] is
  block-parallelized: time is split into 129 tiles of L=128 samples on a grid
  left-padded by 74 zeros (so both passes share one block-aligned grid).
  Per tile: y_tile = Htri @ x_tile + P @ z_tile (Toeplitz matmul + state
  injection). States at all 16 tile starts of a chunk are produced stacked in
  one PSUM tile by 17 accumulating matmuls whose lhsT are column slices of one
  wide constant [zeros | Q0^T]. Pass 1 repartitions the stack to [8, 16*128]
  (via a DRAM bounce) so four tiles' states form one N=512 moving operand;
  pass 2 consumes 32-row slices of the stack directly as stationary operands
  against a block-diagonal constant. The backward pass is the same machinery
  with time-reversed constants, consuming the forward pass's time-major
  output tiles directly (no data flips anywhere).

All constant matrices are built host-side in float64 from the b/a/zi inputs
and stored fp32. Data-parallel across 8 NeuronCores (batch sharding).
"""
import ml_dtypes
import numpy as np
from contextlib import ExitStack

import concourse.bass as bass  # noqa: F401
import concourse.tile as tile
from concourse import bacc, mybir
from concourse.bass_utils import run_bass_kernel_spmd

BATCH, T = 1024, 16384
NCORES = 8
BSH = BATCH // NCORES
L = 128
S = 8
CH = 16
PADLEN = 27
T0 = T + 2 * PADLEN          # 16438
PADL = (-T0) % L             # 74
TP = T0 + PADL               # 16512
NB = TP // L                 # 129
NG = (NB + 3) // 4           # 33 groups of <=4 tiles (last has 1)
NCHUNK = 8                   # full 16-tile chunks; tile 128 rides on chunk 7's tail
F32 = mybir.dt.float32
BF16 = mybir.dt.bfloat16

CONST_SHAPES = {
    "HT1": (L, L), "PT1": (S, L), "QT1w": (L, 2 * L), "SST": (S, L),
    "HT2": (L, L), "QT2w": (L, 2 * L),
    "PG2_0": (L, 512), "PG2_1": (L, 512), "PG2_2": (L, 512), "PG2_3": (L, 512),
    "injZ": (1, L), "injy0": (1, L), "s2z": (1, L), "p2zi": (1, L),
    "ident": (L, L),
}


def _build_const_arrays(b, a, zi):
    """Ascending stacking: stacked row-block rho holds the state entering
    chunk block rho+1 (rho=15 is the chunk-chain output)."""
    b = np.asarray(b, np.float64)
    a = np.asarray(a, np.float64)
    zi = np.asarray(zi, np.float64)
    A = np.zeros((S, S))
    A[:, 0] = -a[1:]
    A[np.arange(S - 1), np.arange(1, S)] = 1.0
    g = b[1:] - a[1:] * b[0]
    Apow = [np.eye(S)]
    for _ in range(L):
        Apow.append(A @ Apow[-1])
    A_L = Apow[L]
    ALp = [np.eye(S)]
    for _ in range(CH + 1):
        ALp.append(A_L @ ALp[-1])
    h = np.zeros(L)
    h[0] = b[0]
    for m in range(1, L):
        h[m] = (Apow[m - 1] @ g)[0]
    W = np.stack([Apow[L - 1 - j] @ g for j in range(L)], axis=1)   # [S, L]
    Htri = np.zeros((L, L))
    for j in range(L):
        Htri[j, :j + 1] = h[j::-1]
    P = np.stack([Apow[j][0, :] for j in range(L)], axis=0)         # [L, S]
    Jf = np.eye(L)[::-1]
    Htri2 = Jf @ Htri @ Jf
    P2 = Jf @ P
    W2 = W @ Jf

    Q0 = np.zeros((L, L))       # row 8*rho+s = (A_L^rho W)[s]
    Q02 = np.zeros((L, L))
    Sst = np.zeros((L, S))      # row 8*rho+s = (A_L^{rho+1})[s]
    injZ = np.zeros(L)          # pass-1 chunk-0 seed: (A_L^rho @ zint)[s]
    s2z = np.zeros(L)           # pass-2 chunk-0 seed: (A_L^{rho+1} zi)[s]
    zint = Apow[L - PADL] @ zi
    for rho in range(CH):
        r0 = rho * S
        Q0[r0:r0 + S, :] = ALp[rho] @ W
        Q02[r0:r0 + S, :] = ALp[rho] @ W2
        Sst[r0:r0 + S, :] = ALp[rho + 1]
        injZ[r0:r0 + S] = ALp[rho] @ zint
        s2z[r0:r0 + S] = ALp[rho + 1] @ zi
    QT1w = np.concatenate([np.zeros((L, L)), Q0.T], axis=1)          # [L, 2L]
    QT2w = np.concatenate([np.zeros((L, L)), Q02.T], axis=1)
    injy0 = np.zeros(L)
    for j in range(PADL, L):
        injy0[j] = (Apow[j - PADL] @ zi)[0]
    p2zi = P2 @ zi
    # PG2_k [128, 512]: contracting the full stacked-state tile against a
    # block-diagonal selector: row 8*rho+sigma feeds out col-slice i iff
    # rho == 4k + 3 - i (ascending rho = descending tile within a group).
    PG2 = [np.zeros((L, 512)) for _ in range(4)]
    for k in range(4):
        for i_out in range(4):
            rho = 4 * k + 3 - i_out
            PG2[k][S * rho:S * (rho + 1), 128 * i_out:128 * (i_out + 1)] = P2.T
    f = ml_dtypes.bfloat16
    return {
        "HT1": np.ascontiguousarray(Htri.T, f), "PT1": np.ascontiguousarray(P.T, f),
        "QT1w": np.ascontiguousarray(QT1w, f), "SST": np.ascontiguousarray(Sst.T, f),
        "HT2": np.ascontiguousarray(Htri2.T, f), "QT2w": np.ascontiguousarray(QT2w, f),
        "PG2_0": PG2[0].astype(f), "PG2_1": PG2[1].astype(f),
        "PG2_2": PG2[2].astype(f), "PG2_3": PG2[3].astype(f),
        "injZ": injZ.reshape(1, L).astype(f), "injy0": injy0.reshape(1, L).astype(f),
        "s2z": s2z.reshape(1, L).astype(f), "p2zi": p2zi.reshape(1, L).astype(f),
        "ident": np.eye(L, dtype=f),
    }


_NC_CACHE = None


def _build_nc():
    nc = bacc.Bacc("TRN2", target_bir_lowering=False, debug=False,
                   num_devices=NCORES)
    xe = nc.dram_tensor("xe", [BSH, TP], BF16, kind="ExternalInput").ap()
    yo = nc.dram_tensor("y", [BSH, T], F32, kind="ExternalOutput").ap()
    cdr = {n: nc.dram_tensor(n, list(s), BF16, kind="ExternalInput").ap()
           for n, s in CONST_SHAPES.items()}

    with tile.TileContext(nc) as tc, ExitStack() as ctx:
        consts = ctx.enter_context(tc.tile_pool(name="consts", bufs=1))
        xstage = ctx.enter_context(tc.tile_pool(name="xstage", bufs=3))
        xtpool = ctx.enter_context(tc.tile_pool(name="xtp", bufs=1))
        y1pool = ctx.enter_context(tc.tile_pool(name="y1p", bufs=1))
        zcpool = ctx.enter_context(tc.tile_pool(name="zcp", bufs=1))
        ystage = ctx.enter_context(tc.tile_pool(name="ystage", bufs=3))
        dstage = ctx.enter_context(tc.tile_pool(name="dstage", bufs=2, space="DRAM"))
        ps_tr = ctx.enter_context(tc.tile_pool(name="ps_tr", bufs=2, space="PSUM"))
        ps_zc = ctx.enter_context(tc.tile_pool(name="ps_zc", bufs=3, space="PSUM"))
        ps_y = ctx.enter_context(tc.tile_pool(name="ps_y", bufs=3, space="PSUM"))

        C = {}
        for n, s in CONST_SHAPES.items():
            t = consts.tile(list(s), BF16, tag=f"c_{n}", name=f"sb_{n}")
            nc.scalar.dma_start(out=t, in_=cdr[n])
            C[n] = t
        # ext[:,0] (= xe col PADL) as a [1, BSH] partition-0 row
        ext0_t = consts.tile([1, BSH], BF16, tag="ext0")
        nc.gpsimd.dma_start(out=ext0_t, in_=xe[:, PADL:PADL + 1].rearrange("b one -> one b"))

        # ---------------- PASS 1 ----------------
        xt_tiles = []
        for gi in range(NG):
            nt = 4 if gi < NG - 1 else 1
            ncol = nt * L
            xs = xstage.tile([BSH, 512], BF16, tag="xs", name=f"xs{gi}")
            nc.sync.dma_start(out=xs[:, :ncol], in_=xe[:, gi * 512:gi * 512 + ncol])
            pt = ps_tr.tile([L, 512], BF16, tag="pt", name=f"pt{gi}")
            for i in range(nt):
                nc.tensor.transpose(pt[:, i * L:(i + 1) * L],
                                    xs[:, i * L:(i + 1) * L], C["ident"])
            xt = xtpool.tile([L, 512], BF16, tag=f"xt{gi}", name=f"xt{gi}")
            nc.scalar.copy(out=xt[:, :ncol], in_=pt[:, :ncol])
            xt_tiles.append(xt)

        def XT(q):
            return xt_tiles[q // 4][:, (q % 4) * L:(q % 4 + 1) * L]

        def chunk_stack(tag, qtw, rhs_tile_fn, seed_lhsT, seed_rhs, zchain):
            """17 accumulating matmuls -> stacked states [128, BSH] in PSUM;
            copies to SBUF and returns (stack, chain) where chain is the
            rows [120:128] repositioned to partition 0 via a small DMA."""
            zp = ps_zc.tile([L, BSH], F32, tag="zp", name=f"zp_{tag}")
            for m in range(CH):
                nc.tensor.matmul(zp, qtw[:, L - S * m:2 * L - S * m],
                                 rhs_tile_fn(m), start=(m == 0), stop=False)
            if zchain is None:
                nc.tensor.matmul(zp, seed_lhsT, seed_rhs, start=False, stop=True)
            else:
                nc.tensor.matmul(zp, C["SST"], zchain, start=False, stop=True)
            zs = zcpool.tile([L, BSH], BF16, tag=f"zs{tag[0]}", bufs=2, name=f"zs_{tag}")
            nc.scalar.copy(out=zs, in_=zp)
            zch = zcpool.tile([S, BSH], BF16, tag=f"zch{tag[0]}", bufs=2, name=f"zch_{tag}")
            nc.gpsimd.dma_start(out=zch, in_=zs[L - S:L, :])
            return zs, zch

        zc1 = []   # pass-1 repartitioned states [8, CH*BSH]: col-block u = state of tile 16c+u+1
        zch1 = []
        for c in range(NCHUNK):
            zchain = None if c == 0 else zch1[c - 1]
            zs, zch = chunk_stack(f"1_{c}", C["QT1w"], lambda m, c=c: XT(16 * c + m),
                                  C["injZ"], ext0_t, zchain)
            zch1.append(zch)
            dz = dstage.tile([CH, S, BSH], BF16, tag="dz", name=f"dz{c}")
            nc.scalar.dma_start(out=dz, in_=zs)
            zr = zcpool.tile([S, CH * BSH], BF16, tag="zr1", bufs=3, name=f"zr1_{c}")
            nc.scalar.dma_start(out=zr.rearrange("s (r b) -> s r b", r=CH),
                                in_=dz.rearrange("r s b -> s r b"))
            zc1.append(zr)

        y1_tiles = [None] * NG

        def emit_y1_group(gi):
            nt = 4 if gi < NG - 1 else 1
            ncol = nt * L
            yp = ps_y.tile([L, 512], F32, tag="yp", name=f"yp1_{gi}")
            nc.tensor.matmul(yp[:, :ncol], C["HT1"], xt_tiles[gi][:, :ncol],
                             start=True, stop=False)
            # states for tiles 4gi..4gi+3 live at u = (q-1) mod 16 in chunk (q-1)//16
            q0 = 4 * gi
            if gi == 0:
                nc.tensor.matmul(yp[:, 0:L], C["injy0"], ext0_t,
                                 start=False, stop=False)
                nc.tensor.matmul(yp[:, L:4 * L], C["PT1"], zc1[0][:, 0:3 * BSH],
                                 start=False, stop=True)
            elif gi == NG - 1:
                nc.tensor.matmul(yp[:, 0:L], C["PT1"],
                                 zc1[7][:, (CH - 1) * BSH:CH * BSH],
                                 start=False, stop=True)
            elif gi % 4 == 0:
                c = gi // 4
                nc.tensor.matmul(yp[:, 0:L], C["PT1"],
                                 zc1[c - 1][:, (CH - 1) * BSH:CH * BSH],
                                 start=False, stop=False)
                nc.tensor.matmul(yp[:, L:4 * L], C["PT1"], zc1[c][:, 0:3 * BSH],
                                 start=False, stop=True)
            else:
                c, u0 = divmod(q0 - 1, CH)
                nc.tensor.matmul(yp[:, :4 * L], C["PT1"],
                                 zc1[c][:, u0 * BSH:(u0 + 4) * BSH],
                                 start=False, stop=True)
            y1 = y1pool.tile([L, 512], BF16, tag=f"y1{gi}", name=f"y1_{gi}")
            nc.vector.tensor_copy(out=y1[:, :ncol], in_=yp[:, :ncol])
            y1_tiles[gi] = y1

        for c in range(NCHUNK):
            for gi in range(4 * c, 4 * c + 4):
                emit_y1_group(gi)
        emit_y1_group(NG - 1)

        def Y1T(q):
            return y1_tiles[q // 4][:, (q % 4) * L:(q % 4 + 1) * L]

        # y1[:, t=16437] as a [1, BSH] partition-0 row (via SBUF->SBUF DMA)
        w0_t = consts.tile([1, BSH], BF16, tag="w0")
        nc.gpsimd.dma_start(out=w0_t, in_=Y1T(NB - 1)[L - 1:L, :])

        # ---------------- PASS 2 (right-to-left) ----------------
        zc2 = []     # stacked SBUF states per chunk [128, BSH]
        zch2 = []    # chain rows [120:128] repositioned to partition 0

        def emit_y2_group(gi):
            # tiles 4gi..4gi+3; their states: chunk c rows [8*rho0, +32),
            # rho0 = 124 - 16c - 4gi, in reversed within-group order (PG2).
            nt = 4 if gi < NG - 1 else 1
            yp = ps_y.tile([BSH, 512], F32, tag="yp", name=f"yp2_{gi}")
            if gi == NG - 1:
                nc.tensor.matmul(yp[:, 0:L], w0_t, C["p2zi"], start=True, stop=False)
                nc.tensor.matmul(yp[:, 0:L], Y1T(NB - 1), C["HT2"],
                                 start=False, stop=True)
            else:
                c = (127 - 4 * gi - 3) // 16
                k = (124 - 16 * c - 4 * gi) // 4
                nc.tensor.matmul(yp, zc2[c], C[f"PG2_{k}"], start=True, stop=False)
                for i in range(nt):
                    q = 4 * gi + i
                    nc.tensor.matmul(yp[:, i * L:(i + 1) * L], Y1T(q), C["HT2"],
                                     start=False, stop=(i == nt - 1))
            ys = ystage.tile([BSH, 512], F32, tag="ys", name=f"ys{gi}")
            nc.vector.tensor_copy(out=ys[:, :nt * L], in_=yp[:, :nt * L])
            # tile q covers y-out index range [128q - 101, 128q + 27)
            if gi == 0:
                nc.scalar.dma_start(out=yo[:, 0:411], in_=ys[:, 101:512])
            elif gi == NG - 1:
                nc.scalar.dma_start(out=yo[:, 512 * gi - 101:T], in_=ys[:, 0:101])
            else:
                nc.scalar.dma_start(out=yo[:, 512 * gi - 101:512 * gi + 411],
                                    in_=ys[:, 0:512])

        emit_y2_group(NG - 1)
        for c in range(NCHUNK):
            zchain = None if c == 0 else zch2[c - 1]
            zs, zch = chunk_stack(f"2_{c}", C["QT2w"],
                                  lambda m, c=c: Y1T(NB - 1 - 16 * c - m),
                                  C["s2z"], w0_t, zchain)
            zc2.append(zs)
            zch2.append(zch)
            for gi in range(31 - 4 * c, 27 - 4 * c, -1):
                emit_y2_group(gi)
    nc.compile()
    return nc


def _host_ext(x):
    x = np.asarray(x, np.float32)
    left = 2.0 * x[:, :1] - x[:, 1:PADLEN + 1][:, ::-1]
    right = 2.0 * x[:, -1:] - x[:, -PADLEN - 1:-1][:, ::-1]
    ext = np.concatenate(
        [np.zeros((x.shape[0], PADL), np.float32), left, x, right], axis=1)
    return ext.astype(ml_dtypes.bfloat16)


def _run(x, b, a, zi, **spmd_kwargs):
    global _NC_CACHE
    if _NC_CACHE is None:
        _NC_CACHE = _build_nc()
    nc = _NC_CACHE
    consts = _build_const_arrays(b, a, zi)
    ext = _host_ext(x)
    in_maps = []
    for i in range(NCORES):
        m = {"xe": np.ascontiguousarray(ext[i * BSH:(i + 1) * BSH])}
        m.update(consts)
        in_maps.append(m)
    res = run_bass_kernel_spmd(nc, in_maps, core_ids=list(range(NCORES)),
                               **spmd_kwargs)
    out = np.concatenate([res.results[i]["y"] for i in range(NCORES)], axis=0)
    return out, res


def kernel(x, b, a, zi):
    return _run(x, b, a, zi)[0]


# revision 26
# speedup vs baseline: 2.2051x; 1.0704x over previous
"""Trainium2 Bass kernel: batched Butterworth bandpass filtfilt (order 4).

Implements y = filtfilt(b, a, x) per batch row (scipy default odd padding),
exactly mirroring the reference: odd-extend by 27, forward IIR pass with
zi*ext[0] initial state, backward (anti-causal) pass with zi*y1[-1], slice.

Algorithm (per core, 128 batch rows in the partition dim):
  The DF2T recurrence z_{t+1} = A z_t + g x_t, y_t = b0 x_t + z_t[0] is
  block-parallelized: time is split into 129 tiles of L=128 samples on a grid
  left-padded by 74 zeros (so both passes share one block-aligned grid).
  Per tile: y_tile = Htri @ x_tile + P @ z_tile (Toeplitz matmul + state
  injection). States at all 16 tile starts of a chunk are produced stacked in
  one PSUM tile by 17 accumulating matmuls whose lhsT are column slices of one
  wide constant [zeros | Q0^T]. Pass 1 repartitions the stack to [8, 16*128]
  (via a DRAM bounce) so four tiles' states form one N=512 moving operand;
  pass 2 consumes 32-row slices of the stack directly as stationary operands
  against a block-diagonal constant. The backward pass is the same machinery
  with time-reversed constants, consuming the forward pass's time-major
  output tiles directly (no data flips anywhere).

All constant matrices are built host-side in float64 from the b/a/zi inputs
and stored fp32. Data-parallel across 8 NeuronCores (batch sharding).
"""
import ml_dtypes
import numpy as np
from contextlib import ExitStack

import concourse.bass as bass  # noqa: F401
import concourse.tile as tile
from concourse import bacc, mybir
from concourse.bass_utils import run_bass_kernel_spmd

BATCH, T = 1024, 16384
NCORES = 8
BSH = BATCH // NCORES
L = 128
S = 8
CH = 16
PADLEN = 27
T0 = T + 2 * PADLEN          # 16438
PADL = (-T0) % L             # 74
TP = T0 + PADL               # 16512
NB = TP // L                 # 129
NG = (NB + 3) // 4           # 33 groups of <=4 tiles (last has 1)
NCHUNK = 8                   # full 16-tile chunks; tile 128 rides on chunk 7's tail
F32 = mybir.dt.float32
BF16 = mybir.dt.bfloat16

CONST_SHAPES = {
    "HT1": (L, L), "PT1": (S, L), "QT1w": (L, 2 * L), "SST": (S, L),
    "HT2": (L, L), "QT2w": (L, 2 * L),
    "PG2_0": (L, 512), "PG2_1": (L, 512), "PG2_2": (L, 512), "PG2_3": (L, 512),
    "injZ": (1, L), "injy0": (1, L), "s2z": (1, L), "p2zi": (1, L),
    "ident": (L, L),
}


def _build_const_arrays(b, a, zi):
    """Ascending stacking: stacked row-block rho holds the state entering
    chunk block rho+1 (rho=15 is the chunk-chain output)."""
    b = np.asarray(b, np.float64)
    a = np.asarray(a, np.float64)
    zi = np.asarray(zi, np.float64)
    A = np.zeros((S, S))
    A[:, 0] = -a[1:]
    A[np.arange(S - 1), np.arange(1, S)] = 1.0
    g = b[1:] - a[1:] * b[0]
    Apow = [np.eye(S)]
    for _ in range(L):
        Apow.append(A @ Apow[-1])
    A_L = Apow[L]
    ALp = [np.eye(S)]
    for _ in range(CH + 1):
        ALp.append(A_L @ ALp[-1])
    h = np.zeros(L)
    h[0] = b[0]
    for m in range(1, L):
        h[m] = (Apow[m - 1] @ g)[0]
    W = np.stack([Apow[L - 1 - j] @ g for j in range(L)], axis=1)   # [S, L]
    Htri = np.zeros((L, L))
    for j in range(L):
        Htri[j, :j + 1] = h[j::-1]
    P = np.stack([Apow[j][0, :] for j in range(L)], axis=0)         # [L, S]
    Jf = np.eye(L)[::-1]
    Htri2 = Jf @ Htri @ Jf
    P2 = Jf @ P
    W2 = W @ Jf

    Q0 = np.zeros((L, L))       # row 8*rho+s = (A_L^rho W)[s]
    Q02 = np.zeros((L, L))
    Sst = np.zeros((L, S))      # row 8*rho+s = (A_L^{rho+1})[s]
    injZ = np.zeros(L)          # pass-1 chunk-0 seed: (A_L^rho @ zint)[s]
    s2z = np.zeros(L)           # pass-2 chunk-0 seed: (A_L^{rho+1} zi)[s]
    zint = Apow[L - PADL] @ zi
    for rho in range(CH):
        r0 = rho * S
        Q0[r0:r0 + S, :] = ALp[rho] @ W
        Q02[r0:r0 + S, :] = ALp[rho] @ W2
        Sst[r0:r0 + S, :] = ALp[rho + 1]
        injZ[r0:r0 + S] = ALp[rho] @ zint
        s2z[r0:r0 + S] = ALp[rho + 1] @ zi
    QT1w = np.concatenate([np.zeros((L, L)), Q0.T], axis=1)          # [L, 2L]
    QT2w = np.concatenate([np.zeros((L, L)), Q02.T], axis=1)
    injy0 = np.zeros(L)
    for j in range(PADL, L):
        injy0[j] = (Apow[j - PADL] @ zi)[0]
    p2zi = P2 @ zi
    # PG2_k [128, 512]: contracting the full stacked-state tile against a
    # block-diagonal selector: row 8*rho+sigma feeds out col-slice i iff
    # rho == 4k + 3 - i (ascending rho = descending tile within a group).
    PG2 = [np.zeros((L, 512)) for _ in range(4)]
    for k in range(4):
        for i_out in range(4):
            rho = 4 * k + 3 - i_out
            PG2[k][S * rho:S * (rho + 1), 128 * i_out:128 * (i_out + 1)] = P2.T
    f = ml_dtypes.bfloat16
    return {
        "HT1": np.ascontiguousarray(Htri.T, f), "PT1": np.ascontiguousarray(P.T, f),
        "QT1w": np.ascontiguousarray(QT1w, f), "SST": np.ascontiguousarray(Sst.T, f),
        "HT2": np.ascontiguousarray(Htri2.T, f), "QT2w": np.ascontiguousarray(QT2w, f),
        "PG2_0": PG2[0].astype(f), "PG2_1": PG2[1].astype(f),
        "PG2_2": PG2[2].astype(f), "PG2_3": PG2[3].astype(f),
        "injZ": injZ.reshape(1, L).astype(f), "injy0": injy0.reshape(1, L).astype(f),
        "s2z": s2z.reshape(1, L).astype(f), "p2zi": p2zi.reshape(1, L).astype(f),
        "ident": np.eye(L, dtype=f),
    }


_NC_CACHE = None


def _build_nc():
    nc = bacc.Bacc("TRN2", target_bir_lowering=False, debug=False,
                   num_devices=NCORES)
    xe = nc.dram_tensor("xe", [BSH, TP], BF16, kind="ExternalInput").ap()
    yo = nc.dram_tensor("y", [BSH, T], F32, kind="ExternalOutput").ap()
    cdr = {n: nc.dram_tensor(n, list(s), BF16, kind="ExternalInput").ap()
           for n, s in CONST_SHAPES.items()}

    with tile.TileContext(nc) as tc, ExitStack() as ctx:
        consts = ctx.enter_context(tc.tile_pool(name="consts", bufs=1))
        xstage = ctx.enter_context(tc.tile_pool(name="xstage", bufs=3))
        xtpool = ctx.enter_context(tc.tile_pool(name="xtp", bufs=1))
        y1pool = ctx.enter_context(tc.tile_pool(name="y1p", bufs=1))
        zcpool = ctx.enter_context(tc.tile_pool(name="zcp", bufs=1))
        ystage = ctx.enter_context(tc.tile_pool(name="ystage", bufs=3))
        dstage = ctx.enter_context(tc.tile_pool(name="dstage", bufs=2, space="DRAM"))
        ps_tr = ctx.enter_context(tc.tile_pool(name="ps_tr", bufs=2, space="PSUM"))
        ps_zc = ctx.enter_context(tc.tile_pool(name="ps_zc", bufs=2, space="PSUM"))
        ps_w = ctx.enter_context(tc.tile_pool(name="ps_w", bufs=1, space="PSUM"))
        ps_y = ctx.enter_context(tc.tile_pool(name="ps_y", bufs=3, space="PSUM"))

        C = {}
        for n, s in CONST_SHAPES.items():
            t = consts.tile(list(s), BF16, tag=f"c_{n}", name=f"sb_{n}")
            nc.scalar.dma_start(out=t, in_=cdr[n])
            C[n] = t
        # ext[:,0] (= xe col PADL) as a [1, BSH] partition-0 row
        ext0_t = consts.tile([1, BSH], BF16, tag="ext0")
        nc.gpsimd.dma_start(out=ext0_t, in_=xe[:, PADL:PADL + 1].rearrange("b one -> one b"))

        # PE clock prewarm: dense dummy matmuls under the initial DMA loads
        # (HAM gate releases to 2.4 GHz after ~3.4 us of sustained activity)
        wp = ps_w.tile([L, 2 * L], F32, tag="wp")
        for _ in range(30):
            nc.tensor.matmul(wp, C["HT1"], C["QT1w"], start=True, stop=True)

        # ---------------- PASS 1 ----------------
        xt_tiles = []
        for gi in range(NG):
            nt = 4 if gi < NG - 1 else 1
            ncol = nt * L
            xs = xstage.tile([BSH, 512], BF16, tag="xs", name=f"xs{gi}")
            nc.sync.dma_start(out=xs[:, :ncol], in_=xe[:, gi * 512:gi * 512 + ncol])
            pt = ps_tr.tile([L, 512], BF16, tag="pt", name=f"pt{gi}")
            for i in range(nt):
                nc.tensor.transpose(pt[:, i * L:(i + 1) * L],
                                    xs[:, i * L:(i + 1) * L], C["ident"])
            xt = xtpool.tile([L, 512], BF16, tag=f"xt{gi}", name=f"xt{gi}")
            nc.scalar.copy(out=xt[:, :ncol], in_=pt[:, :ncol])
            xt_tiles.append(xt)

        def XT(q):
            return xt_tiles[q // 4][:, (q % 4) * L:(q % 4 + 1) * L]

        def chunk_stack(tag, qtw, rhs_tile_fn, seed_lhsT, seed_rhs, zchain):
            """17 accumulating matmuls -> stacked states [128, BSH] in PSUM;
            copies to SBUF and returns (stack, chain) where chain is the
            rows [120:128] repositioned to partition 0 via a small DMA."""
            zp = ps_zc.tile([L, BSH], F32, tag="zp", name=f"zp_{tag}")
            for m in range(CH):
                nc.tensor.matmul(zp, qtw[:, L - S * m:2 * L - S * m],
                                 rhs_tile_fn(m), start=(m == 0), stop=False)
            if zchain is None:
                nc.tensor.matmul(zp, seed_lhsT, seed_rhs, start=False, stop=True)
            else:
                nc.tensor.matmul(zp, C["SST"], zchain, start=False, stop=True)
            zs = zcpool.tile([L, BSH], BF16, tag=f"zs{tag[0]}", bufs=2, name=f"zs_{tag}")
            nc.scalar.copy(out=zs, in_=zp)
            zch = zcpool.tile([S, BSH], BF16, tag=f"zch{tag[0]}", bufs=2, name=f"zch_{tag}")
            nc.gpsimd.dma_start(out=zch, in_=zs[L - S:L, :])
            return zs, zch

        zc1 = []   # pass-1 repartitioned states [8, CH*BSH]: col-block u = state of tile 16c+u+1
        zch1 = []
        for c in range(NCHUNK):
            zchain = None if c == 0 else zch1[c - 1]
            zs, zch = chunk_stack(f"1_{c}", C["QT1w"], lambda m, c=c: XT(16 * c + m),
                                  C["injZ"], ext0_t, zchain)
            zch1.append(zch)
            dz = dstage.tile([CH, S, BSH], BF16, tag="dz", name=f"dz{c}")
            nc.scalar.dma_start(out=dz, in_=zs)
            zr = zcpool.tile([S, CH * BSH], BF16, tag="zr1", bufs=3, name=f"zr1_{c}")
            nc.scalar.dma_start(out=zr.rearrange("s (r b) -> s r b", r=CH),
                                in_=dz.rearrange("r s b -> s r b"))
            zc1.append(zr)

        y1_tiles = [None] * NG

        def emit_y1_group(gi):
            nt = 4 if gi < NG - 1 else 1
            ncol = nt * L
            yp = ps_y.tile([L, 512], F32, tag="yp", name=f"yp1_{gi}")
            nc.tensor.matmul(yp[:, :ncol], C["HT1"], xt_tiles[gi][:, :ncol],
                             start=True, stop=False)
            # states for tiles 4gi..4gi+3 live at u = (q-1) mod 16 in chunk (q-1)//16
            q0 = 4 * gi
            if gi == 0:
                nc.tensor.matmul(yp[:, 0:L], C["injy0"], ext0_t,
                                 start=False, stop=False)
                nc.tensor.matmul(yp[:, L:4 * L], C["PT1"], zc1[0][:, 0:3 * BSH],
                                 start=False, stop=True)
            elif gi == NG - 1:
                nc.tensor.matmul(yp[:, 0:L], C["PT1"],
                                 zc1[7][:, (CH - 1) * BSH:CH * BSH],
                                 start=False, stop=True)
            elif gi % 4 == 0:
                c = gi // 4
                nc.tensor.matmul(yp[:, 0:L], C["PT1"],
                                 zc1[c - 1][:, (CH - 1) * BSH:CH * BSH],
                                 start=False, stop=False)
                nc.tensor.matmul(yp[:, L:4 * L], C["PT1"], zc1[c][:, 0:3 * BSH],
                                 start=False, stop=True)
            else:
                c, u0 = divmod(q0 - 1, CH)
                nc.tensor.matmul(yp[:, :4 * L], C["PT1"],
                                 zc1[c][:, u0 * BSH:(u0 + 4) * BSH],
                                 start=False, stop=True)
            y1 = y1pool.tile([L, 512], BF16, tag=f"y1{gi}", name=f"y1_{gi}")
            nc.vector.tensor_copy(out=y1[:, :ncol], in_=yp[:, :ncol])
            y1_tiles[gi] = y1

        for c in range(NCHUNK):
            for gi in range(4 * c, 4 * c + 4):
                emit_y1_group(gi)
        emit_y1_group(NG - 1)

        def Y1T(q):
            return y1_tiles[q // 4][:, (q % 4) * L:(q % 4 + 1) * L]

        # y1[:, t=16437] as a [1, BSH] partition-0 row (via SBUF->SBUF DMA)
        w0_t = consts.tile([1, BSH], BF16, tag="w0")
        nc.gpsimd.dma_start(out=w0_t, in_=Y1T(NB - 1)[L - 1:L, :])

        # ---------------- PASS 2 (right-to-left) ----------------
        zc2 = []     # stacked SBUF states per chunk [128, BSH]
        zch2 = []    # chain rows [120:128] repositioned to partition 0

        def emit_y2_group(gi):
            # tiles 4gi..4gi+3; their states: chunk c rows [8*rho0, +32),
            # rho0 = 124 - 16c - 4gi, in reversed within-group order (PG2).
            nt = 4 if gi < NG - 1 else 1
            yp = ps_y.tile([BSH, 512], F32, tag="yp", name=f"yp2_{gi}")
            if gi == NG - 1:
                nc.tensor.matmul(yp[:, 0:L], w0_t, C["p2zi"], start=True, stop=False)
                nc.tensor.matmul(yp[:, 0:L], Y1T(NB - 1), C["HT2"],
                                 start=False, stop=True)
            else:
                c = (127 - 4 * gi - 3) // 16
                k = (124 - 16 * c - 4 * gi) // 4
                nc.tensor.matmul(yp, zc2[c], C[f"PG2_{k}"], start=True, stop=False)
                for i in range(nt):
                    q = 4 * gi + i
                    nc.tensor.matmul(yp[:, i * L:(i + 1) * L], Y1T(q), C["HT2"],
                                     start=False, stop=(i == nt - 1))
            ys = ystage.tile([BSH, 512], F32, tag="ys", name=f"ys{gi}")
            nc.vector.tensor_copy(out=ys[:, :nt * L], in_=yp[:, :nt * L])
            # tile q covers y-out index range [128q - 101, 128q + 27)
            if gi == 0:
                nc.scalar.dma_start(out=yo[:, 0:411], in_=ys[:, 101:512])
            elif gi == NG - 1:
                nc.scalar.dma_start(out=yo[:, 512 * gi - 101:T], in_=ys[:, 0:101])
            else:
                nc.scalar.dma_start(out=yo[:, 512 * gi - 101:512 * gi + 411],
                                    in_=ys[:, 0:512])

        emit_y2_group(NG - 1)
        for c in range(NCHUNK):
            zchain = None if c == 0 else zch2[c - 1]
            zs, zch = chunk_stack(f"2_{c}", C["QT2w"],
                                  lambda m, c=c: Y1T(NB - 1 - 16 * c - m),
                                  C["s2z"], w0_t, zchain)
            zc2.append(zs)
            zch2.append(zch)
            for gi in range(31 - 4 * c, 27 - 4 * c, -1):
                emit_y2_group(gi)
    nc.compile()
    return nc


def _host_ext(x):
    x = np.asarray(x, np.float32)
    left = 2.0 * x[:, :1] - x[:, 1:PADLEN + 1][:, ::-1]
    right = 2.0 * x[:, -1:] - x[:, -PADLEN - 1:-1][:, ::-1]
    ext = np.concatenate(
        [np.zeros((x.shape[0], PADL), np.float32), left, x, right], axis=1)
    return ext.astype(ml_dtypes.bfloat16)


def _run(x, b, a, zi, **spmd_kwargs):
    global _NC_CACHE
    if _NC_CACHE is None:
        _NC_CACHE = _build_nc()
    nc = _NC_CACHE
    consts = _build_const_arrays(b, a, zi)
    ext = _host_ext(x)
    in_maps = []
    for i in range(NCORES):
        m = {"xe": np.ascontiguousarray(ext[i * BSH:(i + 1) * BSH])}
        m.update(consts)
        in_maps.append(m)
    res = run_bass_kernel_spmd(nc, in_maps, core_ids=list(range(NCORES)),
                               **spmd_kwargs)
    out = np.concatenate([res.results[i]["y"] for i in range(NCORES)], axis=0)
    return out, res


def kernel(x, b, a, zi):
    return _run(x, b, a, zi)[0]
